# revision 2
# baseline (speedup 1.0000x reference)
"""Multi-head attention (causal) on 8 TRN2 NeuronCores.

Problem: in_features [2, 2048, 1024], 16 heads x 64 dims, causal MHA with
Q/K/V/O projections (no biases), f32 reference.

Sharding: core c = (batch b = c//4, head-group g = c%4); each core computes
its batch element's attention for 4 heads (256 dims of the concat space)
plus the partial O-projection over those 256 v-dims. The host sums the 4
group partials per batch element (the "all-reduce" of the O projection is
done by the host-side unshard, which is numerically identical).

Per-core kernel (all matmul operands bf16, f32 PSUM accumulation):
  inputs (host pre-transposed):
    xT  [1024, 2048]  x[b].T
    wqT/wkT/wvT [1024, 256]   W[g-rows].T
    woT [256, 1024]           Wo[:, g-cols].T
  stage A: QT/KT [256, 2048] = W.T-stationary matmuls; V [2048, 256+ones]
  stage B: per (head, q-chunk of 512): S^T blocks [k128, 512] on PE,
    exp on ScalarE (scale=1/8), causal mask as post-exp 0/1 multiply,
    AV via V_aug^T @ P^T accumulation (ones column gives softmax row-sums
    in PSUM row 64), normalize with reciprocal + partition_broadcast.
  stage C: partial out [2048, 1024] f32 = AO^T-stationary matmuls, DMA out.
"""

import os
import sys

sys.path.insert(0, "/opt/trn_rl_repo")

import numpy as np
import ml_dtypes

import concourse.bass as bass
import concourse.mybir as mybir
from concourse import bacc
from concourse.bass_utils import run_bass_kernel_spmd
from concourse.tile import TileContext

B, S, D = 2, 2048, 1024
H, DK = 16, 64
G = 4                 # head-groups (cores per batch element)
HG = H // G           # heads per group = 4
GD = HG * DK          # 256 group dims
NCORES = 8
KT = D // 128         # 8 contraction tiles for projections
ST = S // 128         # 16 seq tiles
QC = S // 512         # 4 query chunks
SCALE = 1.0 / 8.0     # 1/sqrt(DK)

BF16 = mybir.dt.bfloat16
F32 = mybir.dt.float32

_NC_CACHE = {}


def _build():
    nc = bacc.Bacc(None)
    xT = nc.declare_dram_parameter("xT", [D, S], BF16, isOutput=False)
    wqT = nc.declare_dram_parameter("wqT", [D, GD], BF16, isOutput=False)
    wkT = nc.declare_dram_parameter("wkT", [D, GD], BF16, isOutput=False)
    wvT = nc.declare_dram_parameter("wvT", [D, GD], BF16, isOutput=False)
    woT = nc.declare_dram_parameter("woT", [GD, D], BF16, isOutput=False)
    out = nc.declare_dram_parameter("out", [S, D], F32, isOutput=True)

    with TileContext(nc) as tc:
        with (
            tc.tile_pool(name="persist", bufs=1) as pp,
            tc.tile_pool(name="pt", bufs=4) as pt_pool,
            tc.tile_pool(name="ps_s", bufs=3, space="PSUM") as ps_s_pool,
            tc.tile_pool(name="ps_o", bufs=2, space="PSUM") as ps_o_pool,
            tc.tile_pool(name="ps_p", bufs=2, space="PSUM") as ps_p_pool,
            tc.tile_pool(name="norm", bufs=3) as norm_pool,
        ):
            # ---- persistent SBUF tensors ----
            xT_t = pp.tile([128, KT, S], BF16, tag="xT")
            wqT_t = pp.tile([128, KT, GD], BF16, tag="wqT")
            wkT_t = pp.tile([128, KT, GD], BF16, tag="wkT")
            wvT_t = pp.tile([128, KT, GD], BF16, tag="wvT")
            woT_t = pp.tile([128, GD // 128, D], BF16, tag="woT")
            qT_t = pp.tile([128, GD // 128, S], BF16, tag="qT")
            kT_t = pp.tile([128, GD // 128, S], BF16, tag="kT")
            # V with a ones column per head: [seq-tile, head, 64+1]
            v_t = pp.tile([128, ST, HG, DK + 1], BF16, tag="v")
            aoT_t = pp.tile([128, GD // 128, S], BF16, tag="aoT")
            # sliding 0/1 causal mask: mask01[r, u] = 1 iff u >= r + 384
            mask_t = pp.tile([128, 1024], BF16, tag="mask")

            # ---- input DMAs ----
            for kt in range(KT):
                nc.sync.dma_start(out=xT_t[:, kt, :], in_=xT[kt * 128:(kt + 1) * 128, :])
            for w_t, w_d in ((wqT_t, wqT), (wkT_t, wkT), (wvT_t, wvT)):
                nc.sync.dma_start(
                    out=w_t[:],
                    in_=w_d.rearrange("(t p) n -> p t n", p=128),
                )
            nc.sync.dma_start(
                out=woT_t[:], in_=woT.rearrange("(t p) n -> p t n", p=128)
            )

            # ---- masks / ones init ----
            nc.gpsimd.memset(v_t[:, :, :, DK], 1.0)
            nc.gpsimd.memset(mask_t[:], 1.0)
            # keep where (-1)*r + 1*u + (-384) >= 0, else fill 0
            nc.gpsimd.affine_select(
                out=mask_t[:],
                in_=mask_t[:],
                compare_op=mybir.AluOpType.is_ge,
                fill=0.0,
                base=-384,
                pattern=[[1, 1024]],
                channel_multiplier=-1,
            )

            # ---- stage A: projections ----
            # QT / KT: [256, 2048] = (W slice).T-stationary over xT
            for w_t, dst in ((wqT_t, qT_t), (wkT_t, kT_t)):
                for mt in range(GD // 128):
                    for qc in range(QC):
                        ps = ps_p_pool.tile([128, 512], F32, tag="ps_p")
                        for kt in range(KT):
                            nc.tensor.matmul(
                                ps[:],
                                w_t[:, kt, mt * 128:(mt + 1) * 128],
                                xT_t[:, kt, qc * 512:(qc + 1) * 512],
                                start=(kt == 0),
                                stop=(kt == KT - 1),
                            )
                        nc.scalar.copy(dst[:, mt, qc * 512:(qc + 1) * 512], ps[:])
            # V: [2048, 256] = xT-stationary over wvT, head-strided dst
            for st in range(ST):
                ps = ps_p_pool.tile([128, GD], F32, tag="ps_p")
                for kt in range(KT):
                    nc.tensor.matmul(
                        ps[:],
                        xT_t[:, kt, st * 128:(st + 1) * 128],
                        wvT_t[:, kt, :],
                        start=(kt == 0),
                        stop=(kt == KT - 1),
                    )
                nc.vector.tensor_copy(
                    v_t[:, st, :, 0:DK],
                    ps[:].rearrange("p (h d) -> p h d", h=HG),
                )

            # ---- stage B: attention per (head, q-chunk) ----
            for qc in range(QC):
                n_kt = 4 * qc + 4
                for h in range(HG):
                    mt = h // 2
                    poff = 64 * (h % 2)
                    ps_o = ps_o_pool.tile([128, 512], F32, tag="ps_o")
                    for kt in range(n_kt):
                        ps_s = ps_s_pool.tile([128, 512], F32, tag="ps_s")
                        nc.tensor.matmul(
                            ps_s[:],
                            kT_t[poff:poff + 64, mt, kt * 128:(kt + 1) * 128],
                            qT_t[poff:poff + 64, mt, qc * 512:(qc + 1) * 512],
                            start=True,
                            stop=True,
                        )
                        pt = pt_pool.tile([128, 512], BF16, tag="pt")
                        nc.scalar.activation(
                            pt[:], ps_s[:], mybir.ActivationFunctionType.Exp,
                            scale=SCALE,
                        )
                        if kt >= 4 * qc:
                            j = kt - 4 * qc
                            nc.vector.tensor_mul(
                                pt[:], pt[:],
                                mask_t[:, 384 - 128 * j: 896 - 128 * j],
                            )
                        nc.tensor.matmul(
                            ps_o[0:DK + 1, :],
                            v_t[:, kt, h, :],
                            pt[:],
                            start=(kt == 0),
                            stop=(kt == n_kt - 1),
                        )
                    recip = norm_pool.tile([1, 512], F32, tag="recip")
                    nc.vector.reciprocal(recip[:], ps_o[DK:DK + 1, :])
                    bc = norm_pool.tile([64, 512], F32, tag="bc")
                    nc.gpsimd.partition_broadcast(bc[:], recip[:], channels=64)
                    nc.vector.tensor_mul(
                        aoT_t[poff:poff + 64, mt, qc * 512:(qc + 1) * 512],
                        ps_o[0:DK, :],
                        bc[:],
                    )

            # ---- stage C: partial O projection ----
            for st in range(ST):
                for oc in range(2):
                    ps = ps_p_pool.tile([128, 512], F32, tag="ps_p")
                    for vt in range(GD // 128):
                        nc.tensor.matmul(
                            ps[:],
                            aoT_t[:, vt, st * 128:(st + 1) * 128],
                            woT_t[:, vt, oc * 512:(oc + 1) * 512],
                            start=(vt == 0),
                            stop=(vt == GD // 128 - 1),
                        )
                    osb = norm_pool.tile([128, 512], F32, tag="osb", bufs=3)
                    nc.vector.tensor_copy(osb[:], ps[:])
                    nc.sync.dma_start(
                        out=out[st * 128:(st + 1) * 128, oc * 512:(oc + 1) * 512],
                        in_=osb[:],
                    )

    nc.compile()
    return nc


def _get_nc():
    if "nc" not in _NC_CACHE:
        _NC_CACHE["nc"] = _build()
    return _NC_CACHE["nc"]


def _shard(in_features, q_proj_weight, k_proj_weight, v_proj_weight, o_proj_weight):
    x = np.asarray(in_features, dtype=np.float32)
    wq = np.asarray(q_proj_weight, dtype=np.float32)
    wk = np.asarray(k_proj_weight, dtype=np.float32)
    wv = np.asarray(v_proj_weight, dtype=np.float32)
    wo = np.asarray(o_proj_weight, dtype=np.float32)
    bf = ml_dtypes.bfloat16
    in_maps = []
    for c in range(NCORES):
        b, g = c // G, c % G
        sl = slice(g * GD, (g + 1) * GD)
        in_maps.append({
            "xT": np.ascontiguousarray(x[b].T).astype(bf),
            "wqT": np.ascontiguousarray(wq[sl, :].T).astype(bf),
            "wkT": np.ascontiguousarray(wk[sl, :].T).astype(bf),
            "wvT": np.ascontiguousarray(wv[sl, :].T).astype(bf),
            "woT": np.ascontiguousarray(wo[:, sl].T).astype(bf),
        })
    return in_maps


def _run(trace, **inputs):
    in_maps = _shard(**inputs)
    nc = _get_nc()
    res = run_bass_kernel_spmd(nc, in_maps, core_ids=list(range(NCORES)), trace=trace)
    full = np.zeros((B, S, D), dtype=np.float32)
    for c in range(NCORES):
        full[c // G] += res.results[c]["out"]
    return full, res


def kernel(**inputs):
    full, _ = _run(False, **inputs)
    return full


def kernel_traced(**inputs):
    """Used by test.py: returns (output, BassKernelResults with exec_time_ns)."""
    _install_profiling()
    return _run(True, **inputs)


def _install_profiling():
    import types
    import concourse.bass_utils as bu

    if "antenv.axon_hooks" in sys.modules:
        return
    import trn_agent_boot.trn_boot as tb

    mod = types.ModuleType("antenv.axon_hooks")
    _h = {}
    mod.set_axon_ntff_profile_hook = lambda h: _h.__setitem__("h", h)
    mod.get_axon_ntff_profile_hook = lambda: _h.get("h")
    sys.modules["antenv.axon_hooks"] = mod
    mod.set_axon_ntff_profile_hook(
        tb._ntff_profile_via_ctypes("/opt/axon/libaxon_pjrt.so")
    )
    bu.upload_artifacts = lambda tmpdir: "local://" + tmpdir


# revision 4
# speedup vs baseline: 1.1217x; 1.1217x over previous
"""Multi-head attention (causal) on 8 TRN2 NeuronCores.

Problem: in_features [2, 2048, 1024], 16 heads x 64 dims, causal MHA with
Q/K/V/O projections (no biases), f32 reference.

Sharding: core c = (batch b = c//4, head-group g = c%4); each core computes
its batch element's attention for 4 heads (256 dims of the concat space)
plus the partial O-projection over those 256 v-dims. The host sums the 4
group partials per batch element (the "all-reduce" of the O projection is
done by the host-side unshard, which is numerically identical).

Per-core kernel (all matmul operands bf16, f32 PSUM accumulation):
  inputs (host pre-transposed):
    xT  [1024, 2048]  x[b].T
    wqT/wkT/wvT [1024, 256]   W[g-rows].T
    woT [256, 1024]           Wo[:, g-cols].T
  stage A: QT/KT [256, 2048] = W.T-stationary matmuls; V [2048, 256+ones]
  stage B: per (head, q-chunk of 512): S^T blocks [k128, 512] on PE,
    exp on ScalarE (scale=1/8), causal mask as post-exp 0/1 multiply,
    AV via V_aug^T @ P^T accumulation (ones column gives softmax row-sums
    in PSUM row 64), normalize with reciprocal + partition_broadcast.
  stage C: partial out [2048, 1024] f32 = AO^T-stationary matmuls, DMA out.
"""

import os
import sys

sys.path.insert(0, "/opt/trn_rl_repo")

import numpy as np
import ml_dtypes

import concourse.bass as bass
import concourse.mybir as mybir
from concourse import bacc
from concourse.bass_utils import run_bass_kernel_spmd
from concourse.tile import TileContext

B, S, D = 2, 2048, 1024
H, DK = 16, 64
G = 4                 # head-groups (cores per batch element)
HG = H // G           # heads per group = 4
GD = HG * DK          # 256 group dims
NCORES = 8
KT = D // 128         # 8 contraction tiles for projections
ST = S // 128         # 16 seq tiles
QC = S // 512         # 4 query chunks
SCALE = 1.0 / 8.0     # 1/sqrt(DK)

BF16 = mybir.dt.bfloat16
F32 = mybir.dt.float32

_NC_CACHE = {}


def _build():
    nc = bacc.Bacc(None)
    xT = nc.declare_dram_parameter("xT", [D, S], BF16, isOutput=False)
    wqT = nc.declare_dram_parameter("wqT", [D, GD], BF16, isOutput=False)
    wkT = nc.declare_dram_parameter("wkT", [D, GD], BF16, isOutput=False)
    wvT = nc.declare_dram_parameter("wvT", [D, GD], BF16, isOutput=False)
    woT = nc.declare_dram_parameter("woT", [GD, D], BF16, isOutput=False)
    out = nc.declare_dram_parameter("out", [S, D], F32, isOutput=True)

    with TileContext(nc) as tc:
        with (
            tc.tile_pool(name="persist", bufs=1) as pp,
            tc.tile_pool(name="pt", bufs=4) as pt_pool,
            tc.tile_pool(name="ps_s", bufs=2, space="PSUM") as ps_s_pool,
            tc.tile_pool(name="ps_o", bufs=2, space="PSUM") as ps_o_pool,
            tc.tile_pool(name="ps_p", bufs=2, space="PSUM") as ps_p_pool,
            tc.tile_pool(name="norm", bufs=3) as norm_pool,
        ):
            # ---- persistent SBUF tensors ----
            xT_t = pp.tile([128, KT, S], BF16, tag="xT")
            wqT_t = pp.tile([128, KT, GD], BF16, tag="wqT")
            wkT_t = pp.tile([128, KT, GD], BF16, tag="wkT")
            wvT_t = pp.tile([128, KT, GD], BF16, tag="wvT")
            woT_t = pp.tile([128, GD // 128, D], BF16, tag="woT")
            qT_t = pp.tile([128, GD // 128, S], BF16, tag="qT")
            kT_t = pp.tile([128, GD // 128, S], BF16, tag="kT")
            # V with a ones column per head: [seq-tile, head, 64+1]
            v_t = pp.tile([128, ST, HG, DK + 1], BF16, tag="v")
            aoT_t = pp.tile([128, GD // 128, S], BF16, tag="aoT")
            # sliding 0/1 causal mask: mask01[r, u] = 1 iff u >= r + 384
            mask_t = pp.tile([128, 1024], BF16, tag="mask")

            # ---- input DMAs (weights first: first matmuls need wqT + xT) ----
            for w_t, w_d in ((wqT_t, wqT), (wkT_t, wkT), (wvT_t, wvT)):
                nc.sync.dma_start(
                    out=w_t[:],
                    in_=w_d.rearrange("(t p) n -> p t n", p=128),
                )
            for kt in range(KT):
                nc.sync.dma_start(out=xT_t[:, kt, :], in_=xT[kt * 128:(kt + 1) * 128, :])
            nc.sync.dma_start(
                out=woT_t[:], in_=woT.rearrange("(t p) n -> p t n", p=128)
            )

            # ---- masks / ones init ----
            nc.gpsimd.memset(v_t[:, :, :, DK], 1.0)
            nc.gpsimd.memset(mask_t[:], 1.0)
            # keep where (-1)*r + 1*u + (-384) >= 0, else fill 0
            nc.gpsimd.affine_select(
                out=mask_t[:],
                in_=mask_t[:],
                compare_op=mybir.AluOpType.is_ge,
                fill=0.0,
                base=-384,
                pattern=[[1, 1024]],
                channel_multiplier=-1,
            )

            # ---- stage A: projections ----
            # QT / KT: [256, 2048] = (W slice).T-stationary over xT
            for w_t, dst in ((wqT_t, qT_t), (wkT_t, kT_t)):
                for mt in range(GD // 128):
                    for qc in range(QC):
                        ps = ps_p_pool.tile([128, 512], F32, tag="ps_p")
                        for kt in range(KT):
                            nc.tensor.matmul(
                                ps[:],
                                w_t[:, kt, mt * 128:(mt + 1) * 128],
                                xT_t[:, kt, qc * 512:(qc + 1) * 512],
                                start=(kt == 0),
                                stop=(kt == KT - 1),
                            )
                        nc.vector.tensor_copy(dst[:, mt, qc * 512:(qc + 1) * 512], ps[:])
            # V: [2048, 256] = xT-stationary over wvT, head-strided dst
            for st in range(ST):
                ps = ps_p_pool.tile([128, GD], F32, tag="ps_p")
                for kt in range(KT):
                    nc.tensor.matmul(
                        ps[:],
                        xT_t[:, kt, st * 128:(st + 1) * 128],
                        wvT_t[:, kt, :],
                        start=(kt == 0),
                        stop=(kt == KT - 1),
                    )
                nc.vector.tensor_copy(
                    v_t[:, st, :, 0:DK],
                    ps[:].rearrange("p (h d) -> p h d", h=HG),
                )

            # ---- stage B: attention per (head, q-chunk) ----
            for qc in range(QC):
                n_kt = 4 * qc + 4
                for h in range(HG):
                    mt = h // 2
                    poff = 64 * (h % 2)
                    ps_o = ps_o_pool.tile([128, 512], F32, tag="ps_o")
                    for kp in range(n_kt // 2):
                        # two k-tiles of scores share one PSUM tile -> one exp
                        ps_s = ps_s_pool.tile([128, 1024], F32, tag="ps_s")
                        for u in range(2):
                            kt = 2 * kp + u
                            nc.tensor.matmul(
                                ps_s[:, u * 512:(u + 1) * 512],
                                kT_t[poff:poff + 64, mt, kt * 128:(kt + 1) * 128],
                                qT_t[poff:poff + 64, mt, qc * 512:(qc + 1) * 512],
                                start=True,
                                stop=True,
                            )
                        pt = pt_pool.tile([128, 1024], BF16, tag="pt")
                        nc.scalar.activation(
                            pt[:], ps_s[:], mybir.ActivationFunctionType.Exp,
                            scale=SCALE,
                        )
                        for u in range(2):
                            kt = 2 * kp + u
                            if kt >= 4 * qc:
                                j = kt - 4 * qc
                                nc.vector.tensor_mul(
                                    pt[:, u * 512:(u + 1) * 512],
                                    pt[:, u * 512:(u + 1) * 512],
                                    mask_t[:, 384 - 128 * j: 896 - 128 * j],
                                )
                            nc.tensor.matmul(
                                ps_o[0:DK + 1, :],
                                v_t[:, kt, h, :],
                                pt[:, u * 512:(u + 1) * 512],
                                start=(kt == 0),
                                stop=(kt == n_kt - 1),
                            )
                    recip = norm_pool.tile([1, 512], F32, tag="recip")
                    nc.vector.reciprocal(recip[:], ps_o[DK:DK + 1, :])
                    bc = norm_pool.tile([64, 512], F32, tag="bc")
                    nc.gpsimd.partition_broadcast(bc[:], recip[:], channels=64)
                    nc.vector.tensor_mul(
                        aoT_t[poff:poff + 64, mt, qc * 512:(qc + 1) * 512],
                        ps_o[0:DK, :],
                        bc[:],
                    )

            # ---- stage C: partial O projection ----
            for st in range(ST):
                for oc in range(2):
                    ps = ps_p_pool.tile([128, 512], F32, tag="ps_p")
                    for vt in range(GD // 128):
                        nc.tensor.matmul(
                            ps[:],
                            aoT_t[:, vt, st * 128:(st + 1) * 128],
                            woT_t[:, vt, oc * 512:(oc + 1) * 512],
                            start=(vt == 0),
                            stop=(vt == GD // 128 - 1),
                        )
                    osb = norm_pool.tile([128, 512], F32, tag="osb", bufs=3)
                    nc.vector.tensor_copy(osb[:], ps[:])
                    nc.sync.dma_start(
                        out=out[st * 128:(st + 1) * 128, oc * 512:(oc + 1) * 512],
                        in_=osb[:],
                    )

    nc.compile()
    return nc


def _get_nc():
    if "nc" not in _NC_CACHE:
        _NC_CACHE["nc"] = _build()
    return _NC_CACHE["nc"]


def _shard(in_features, q_proj_weight, k_proj_weight, v_proj_weight, o_proj_weight):
    x = np.asarray(in_features, dtype=np.float32)
    wq = np.asarray(q_proj_weight, dtype=np.float32)
    wk = np.asarray(k_proj_weight, dtype=np.float32)
    wv = np.asarray(v_proj_weight, dtype=np.float32)
    wo = np.asarray(o_proj_weight, dtype=np.float32)
    bf = ml_dtypes.bfloat16
    in_maps = []
    for c in range(NCORES):
        b, g = c // G, c % G
        sl = slice(g * GD, (g + 1) * GD)
        in_maps.append({
            "xT": np.ascontiguousarray(x[b].T).astype(bf),
            "wqT": np.ascontiguousarray(wq[sl, :].T).astype(bf),
            "wkT": np.ascontiguousarray(wk[sl, :].T).astype(bf),
            "wvT": np.ascontiguousarray(wv[sl, :].T).astype(bf),
            "woT": np.ascontiguousarray(wo[:, sl].T).astype(bf),
        })
    return in_maps


def _run(trace, **inputs):
    in_maps = _shard(**inputs)
    nc = _get_nc()
    res = run_bass_kernel_spmd(nc, in_maps, core_ids=list(range(NCORES)), trace=trace)
    full = np.zeros((B, S, D), dtype=np.float32)
    for c in range(NCORES):
        full[c // G] += res.results[c]["out"]
    return full, res


def kernel(**inputs):
    full, _ = _run(False, **inputs)
    return full


def kernel_traced(**inputs):
    """Used by test.py: returns (output, BassKernelResults with exec_time_ns)."""
    _install_profiling()
    return _run(True, **inputs)


def _install_profiling():
    import types
    import concourse.bass_utils as bu

    if "antenv.axon_hooks" in sys.modules:
        return
    import trn_agent_boot.trn_boot as tb

    mod = types.ModuleType("antenv.axon_hooks")
    _h = {}
    mod.set_axon_ntff_profile_hook = lambda h: _h.__setitem__("h", h)
    mod.get_axon_ntff_profile_hook = lambda: _h.get("h")
    sys.modules["antenv.axon_hooks"] = mod
    mod.set_axon_ntff_profile_hook(
        tb._ntff_profile_via_ctypes("/opt/axon/libaxon_pjrt.so")
    )
    bu.upload_artifacts = lambda tmpdir: "local://" + tmpdir
